# revision 26
# baseline (speedup 1.0000x reference)
"""Trainium2 Bass kernel for an autoregressive-flow (MAF) layer.

Reference computation (per region r, batch-network b):
    xr[n, d]   = x[n, region_idx[r, d]]                      # [N, D]
    h1 = relu(xr @ (W1*M1)[r,b])                             # [N, H]
    h2 = relu(h1 @ (W2*M2)[r,b])                             # [N, H]
    o  = h2 @ (W3*M3)[r,b]                                   # [N, 2D]
    shift = o[:, 0::2]; log_scale = o[:, 1::2]
    u  = (xr - shift) * exp(-log_scale)
    ll[n, r, b] = sum_d(-0.5*u^2 - 0.5*log(2*pi) - log_scale)

Sharding: region axis R=8 across the 8 NeuronCores; each core handles its
region's B=16 networks over all N=2048 samples.

Device dataflow (per core, "transposed" orientation), v3:
    - Masks are folded into the weights on the host (M is 0/1 so
      bf16(W*M) == bf16(W)*M exactly); the wall tensor carries only the
      masked weights -> half the weight DMA and no on-device mask pass.
    - HAM warmup: ~20 short fp32 matmuls on a memset tile issue as the
      very first PE work (during the input DMA) so the PE clock gate is
      at 2.4 GHz (K=8/8) before the first real matmul.
    - xtb [128, 2048] bf16: x-slice transposed, replicated on 4 partition
      row-groups (feeds 4x row-packed K=32 L1 matmuls + the seed matmul).
    - Per (chunk, group) work item: L1 (4 row-packed K=32 matmuls),
      relu-moves to bf16 SBUF split across ACT/DVE, L2 (4 full matmuls),
      relu-moves, L3 split into shift/log_scale halves with 4 networks
      column-packed per PSUM tile; shift tile seeded with -x via a
      negated tiled-identity matmul so PSUM holds (shift - x).
    - Tail: A = 0.5*(shift-x)^2 [ACT Square, scale=sqrt(0.5), bf16],
      B = exp(-2*ls) [ACT Exp, bf16], c = A*B [Pool bf16],
      v = c + ls [DVE -> fp32r].  One ll matmul per group (-1 block
      weights, fp32r) accumulates -(0.5*u^2 + ls) into a [16, 512] PSUM
      tile; bias add on DVE.
    - Emission of L3/ll is staggered one work item behind L1/L2 so the
      PE FIFO never head-of-line blocks on the relu-move/tail chains.
"""

import ml_dtypes
import numpy as np

import concourse.bacc as bacc
import concourse.mybir as mybir
from concourse.bass_utils import run_bass_kernel_spmd
from concourse.tile import TileContext

R, B, D, H, N, F = 8, 16, 32, 128, 2048, 256
HALF_LOG_2PI = 0.9189385332046727
N_CORES = 8
CHUNK = 512
F32 = mybir.dt.float32
F32R = mybir.dt.float32r
BF16 = mybir.dt.bfloat16

# wall column layout (bf16): [neg(128) | 4 x group-block(768)]
#   group-block: w2(512) w3(256)   (masks pre-folded on host; W1 ships
#   separately as fp8 hi/lo pairs for DoubleRow matmuls)
GBLK = 768
WALL_C = 128 + 4 * GBLK
F8 = mybir.dt.float8e4
SQRT_HALF = float(np.sqrt(0.5))
N_WARMUP = 9


def _neg_block():
    # Negated tiled identity: lhsT [32, 128], out rows 32*bp + d get -x_d.
    neg = np.zeros((128, 128), np.float32)
    for m in range(128):
        neg[m % D, m] = -1.0
    return neg


def _llw():
    # ll weights [128, 4, 16] fp32: for group g, col j = 4g+bp sums rows
    # 32bp..32bp+31 with -1 (v already holds 0.5*u^2 + ls).
    w = np.zeros((128, 4, 16), np.float32)
    for g in range(4):
        for bp in range(4):
            w[32 * bp : 32 * (bp + 1), g, 4 * g + bp] = -1.0
    return w.reshape(128, 64)


def build_nc(n_total=N):
    assert n_total % CHUNK == 0
    n_chunks = n_total // CHUNK
    n_items = 4 * n_chunks  # (chunk, group) work items

    nc = bacc.Bacc(
        "TRN2",
        target_bir_lowering=False,
        debug=False,
        enable_asserts=False,
        num_devices=N_CORES,
    )

    wall_d = nc.declare_dram_parameter("wall", [128, WALL_C], BF16, isOutput=False)
    llw_d = nc.declare_dram_parameter("llw", [128, 64], F32, isOutput=False)
    w18_d = nc.declare_dram_parameter("w18", [128, 8, 2, 128], F8, isOutput=False)
    xt8_d = nc.declare_dram_parameter("xt8", [128, 2, n_total], F8, isOutput=False)
    xts_d = nc.declare_dram_parameter("xts", [D, n_total], BF16, isOutput=False)
    out_d = nc.declare_dram_parameter("out", [n_chunks, 16, CHUNK], F32, isOutput=True)

    with TileContext(nc) as tc:
        with (
            tc.tile_pool(name="const", bufs=1) as cpool,
            tc.tile_pool(name="stage", bufs=1) as spool,
            tc.tile_pool(name="s1", bufs=8) as s1pool,
            tc.tile_pool(name="s2", bufs=8) as s2pool,
            tc.tile_pool(name="tail", bufs=2) as tpool,
            tc.tile_pool(name="vout", bufs=2) as vpool,
            tc.tile_pool(name="llo", bufs=2) as opool,
            tc.tile_pool(name="p1", bufs=3, space="PSUM") as p1pool,
            tc.tile_pool(name="p2", bufs=2, space="PSUM") as p2pool,
            tc.tile_pool(name="pt", bufs=1, space="PSUM") as ptpool,
            tc.tile_pool(name="pl", bufs=1, space="PSUM") as plpool,
            tc.tile_pool(name="pll", bufs=1, space="PSUM") as pllpool,
        ):
            wall = cpool.tile([128, WALL_C], BF16, tag="wall")
            llwst = spool.tile([128, 64], F32, tag="llwst")
            llwr = cpool.tile([128, 64], F32R, tag="llwr")
            w18 = cpool.tile([128, 8, 2, 128], F8, tag="w18")
            xt8 = cpool.tile([128, 2, n_total], F8, tag="xt8")
            xtb = cpool.tile([D, n_total], BF16, tag="xtb")
            wtile = cpool.tile([128, 64], F32, tag="warm")

            # HAM warmup: memset a small tile, then back-to-back short fp32
            # matmuls keep the PE busy from ~6.3us (while input DMA runs) so
            # the clock gate un-throttles before the first real matmul.
            ftile = cpool.tile([128, 256], BF16, tag="fill")
            nc.gpsimd.memset(wtile[:], 0.0)
            nc.gpsimd.memset(ftile[:], 0.0)
            # Warmup/filler matmuls write a scratch tile in the pll bank: that
            # bank has no real writes until the first ll matmul (emitted at
            # loop i=1), and that matmul's start=True overwrites the region.
            wps = pllpool.tile([16, CHUNK], F32, name="warmps", tag="llps")
            for _ in range(N_WARMUP):
                nc.tensor.matmul(
                    wps[0:16, 0:64],
                    wtile[:, 0:16],
                    wtile[:, 0:64],
                    start=True,
                    stop=True,
                )

            def emit_fill(n):
                # Short bf16 matmuls with no data deps: they run whenever the
                # PE would otherwise idle waiting on relu-moves, keeping the
                # HAM activity window busy during early pipe-fill so the clock
                # gate stays at K=8/8.  Only legal before the first emit_ll.
                for _ in range(n):
                    nc.tensor.matmul(
                        wps[0:16, 0:256],
                        ftile[:, 0:16],
                        ftile[:, 0:256],
                        start=True,
                        stop=True,
                    )

            # Input DMA spread across engine queues so the transfers run in
            # parallel (a single queue moves only ~90 GB/s): the first-needed
            # pieces (neg + L1 g0 weights, x chunk 0, rest of g0) go on their
            # own queues; bulk weights and x chunks 1-3 follow on sync.
            nc.scalar.dma_start(out=w18[:, 0:2], in_=w18_d[:, 0:2])
            nc.sync.dma_start(out=xt8[:, :, :CHUNK], in_=xt8_d[:, :, :CHUNK])
            nc.scalar.dma_start(out=wall[:, :896], in_=wall_d[:, :896])
            nc.sync.dma_start(out=xtb[:, :CHUNK], in_=xts_d[:, :CHUNK])
            nc.sync.dma_start(out=llwst[:], in_=llw_d[:])
            nc.scalar.dma_start(out=w18[:, 2:8], in_=w18_d[:, 2:8])
            nc.scalar.dma_start(out=wall[:, 896:], in_=wall_d[:, 896:])
            nc.sync.dma_start(out=xt8[:, :, CHUNK:], in_=xt8_d[:, :, CHUNK:])
            nc.sync.dma_start(out=xtb[:, CHUNK:], in_=xts_d[:, CHUNK:])

            neg = wall[0:D, 0:128]

            nc.vector.tensor_copy(out=llwr[:], in_=llwst[:])

            def w2m(b):
                g, bp = b // 4, b % 4
                base = 128 + g * GBLK + 128 * bp
                return wall[:, base : base + 128]

            def w3m(b, half):
                # half 0 = shift cols, 1 = log-scale cols
                g, bp = b // 4, b % 4
                base = 128 + g * GBLK + 512 + 64 * bp + 32 * half
                return wall[:, base : base + 32]

            # Per-item state carried between staggered emission phases.
            st = [None] * n_items

            def item_cg(i):
                return i // 4, i % 4

            def emit_L1(i, bps):
                # fp8 DoubleRow with exact hi/lo compensation: contraction
                # runs over [32 rows x (W_hi,W_lo) pairs x x_hi] stacked with
                # [32 rows x (W_hi,0) pairs x x_lo] -> (W_hi+W_lo)x_hi +
                # W_hi x_lo at half the streaming cycles of bf16.
                c, g = item_cg(i)
                cs = slice(c * CHUNK, (c + 1) * CHUNK)
                s = st[i]
                for bp in bps:
                    rb, cb = bp % 2, bp // 2
                    rows = slice(64 * rb, 64 * rb + 64)
                    p1 = p1pool.tile([128, CHUNK], F32, tag="p1")
                    nc.tensor.matmul(
                        p1[:],
                        w18[rows, 2 * g + cb, :, :],
                        xt8[rows, :, cs],
                        start=True,
                        stop=True,
                        perf_mode=mybir.MatmulPerfMode.DoubleRow,
                        tile_position=(64 * rb, 0),
                    )
                    s["p1"][bp] = p1

            def emit_s1_moves(i):
                # bp -> engine: [ACT, DVE, ACT, DVE]
                s = st[i]
                for bp in range(4):
                    s1 = s1pool.tile([128, CHUNK], BF16, tag="s1")
                    p1 = s["p1"][bp]
                    if bp in (0, 2):
                        nc.scalar.activation(
                            s1[:], p1[:], mybir.ActivationFunctionType.Relu
                        )
                    else:
                        nc.vector.tensor_scalar_max(s1[:], p1[:], 0.0)
                    s["s1"][bp] = s1

            def emit_L2(i):
                c, g = item_cg(i)
                s = st[i]
                for bp in range(4):
                    b = 4 * g + bp
                    p2 = p2pool.tile([128, CHUNK], F32, tag="p2")
                    nc.tensor.matmul(
                        p2[:],
                        w2m(b),
                        s["s1"][bp][:],
                        start=True,
                        stop=True,
                    )
                    s["p2"][bp] = p2

            def emit_s2_moves(i):
                # bp -> engine: [DVE, ACT, DVE, ACT]
                s = st[i]
                for bp in range(4):
                    s2 = s2pool.tile([128, CHUNK], BF16, tag="s2")
                    p2 = s["p2"][bp]
                    if bp in (1, 3):
                        nc.scalar.activation(
                            s2[:], p2[:], mybir.ActivationFunctionType.Relu
                        )
                    else:
                        nc.vector.tensor_scalar_max(s2[:], p2[:], 0.0)
                    s["s2"][bp] = s2

            def emit_L3_shift(i):
                c, g = item_cg(i)
                cs = slice(c * CHUNK, (c + 1) * CHUNK)
                s = st[i]
                tps = ptpool.tile([128, CHUNK], F32, tag="tps")
                nc.tensor.matmul(
                    tps[:],
                    neg,
                    xtb[0:D, cs],
                    start=True,
                    stop=False,
                    skip_group_check=True,
                    tile_position=(0, 0),
                )
                for bp in range(4):
                    b = 4 * g + bp
                    nc.tensor.matmul(
                        tps[32 * bp : 32 * (bp + 1), :],
                        w3m(b, 0),
                        s["s2"][bp][:],
                        start=False,
                        stop=(bp == 3),
                        skip_group_check=True,
                        tile_position=(0, 32 * bp),
                    )
                s["tps"] = tps

            def emit_L3_ls(i, bps):
                c, g = item_cg(i)
                s = st[i]
                if s.get("lps") is None:
                    s["lps"] = plpool.tile([128, CHUNK], F32, name="lps", tag="lps")
                lps = s["lps"]
                for bp in bps:
                    b = 4 * g + bp
                    nc.tensor.matmul(
                        lps[32 * bp : 32 * (bp + 1), :],
                        w3m(b, 1),
                        s["s2"][bp][:],
                        start=True,
                        stop=True,
                        tile_position=(0, 32 * bp),
                    )

            def emit_tail(i):
                s = st[i]
                a_sb = tpool.tile([128, CHUNK], BF16, tag="a")
                nc.scalar.activation(
                    a_sb[:],
                    s["tps"][:],
                    mybir.ActivationFunctionType.Square,
                    scale=SQRT_HALF,
                )
                b_sb = tpool.tile([128, CHUNK], BF16, tag="b")
                nc.scalar.activation(
                    b_sb[:],
                    s["lps"][:],
                    mybir.ActivationFunctionType.Exp,
                    scale=-2.0,
                )
                c_sb = tpool.tile([128, CHUNK], BF16, tag="c")
                nc.vector.tensor_mul(out=c_sb[:], in0=a_sb[:], in1=b_sb[:])
                v = vpool.tile([128, CHUNK], F32R, tag="v")
                nc.vector.tensor_add(out=v[:], in0=c_sb[:], in1=s["lps"][:])
                s["v"] = v

            llps_ref = [None]

            def emit_ll(i):
                c, g = item_cg(i)
                if g == 0:
                    llps_ref[0] = pllpool.tile([16, CHUNK], F32, name="llps", tag="llps")
                llps = llps_ref[0]
                nc.tensor.matmul(
                    llps[:],
                    llwr[:, 16 * g : 16 * (g + 1)],
                    st[i]["v"][:],
                    start=(g == 0),
                    stop=(g == 3),
                    skip_group_check=True,
                )
                if g == 3:
                    ll_sb = opool.tile([16, CHUNK], F32, tag="ll")
                    nc.scalar.activation(
                        ll_sb[:],
                        llps[:],
                        mybir.ActivationFunctionType.Copy,
                        bias=float(-D * HALF_LOG_2PI),
                    )
                    nc.sync.dma_start(out=out_d[c], in_=ll_sb[:])
                st[i]["v"] = None
                st[i] = None

            # Staggered emission: PE slot i runs item i's L3 + item (i-1)'s
            # ll interleaved with item (i+1)'s L1/L2, ordered so the PE FIFO
            # never head-of-line blocks on relu-move/tail chains or PSUM
            # bank reuse (p1/p2 are double-buffered; each matmul that reuses
            # a bank issues well after the relu move that frees it).
            def start_item(i):
                st[i] = {"p1": [None] * 4, "s1": [None] * 4,
                         "p2": [None] * 4, "s2": [None] * 4, "lps": None}

            start_item(0)
            emit_L1(0, range(4))
            emit_fill(8)
            emit_s1_moves(0)
            emit_L2(0)
            emit_fill(4)
            emit_s2_moves(0)
            for i in range(n_items):
                nxt = i + 1
                if nxt < n_items:
                    start_item(nxt)
                    emit_L1(nxt, [0, 1, 2])
                if i == 0:
                    emit_fill(3)
                emit_L3_shift(i)
                if i == 0:
                    emit_fill(3)
                if nxt < n_items:
                    emit_L1(nxt, [3])
                    emit_s1_moves(nxt)
                emit_L3_ls(i, [0, 1, 2, 3])
                emit_tail(i)
                if i >= 1:
                    emit_ll(i - 1)
                if nxt < n_items:
                    emit_L2(nxt)
                    emit_s2_moves(nxt)
                if i == 0:
                    emit_fill(3)
            emit_ll(n_items - 1)

    nc.compile()
    return nc


def shard_inputs(x, W1, W2, W3, M1, M2, M3, region_idx, n_total=N):
    """Per-core input dicts: pure gather/transpose/replicate layout prep."""
    x = np.asarray(x, dtype=np.float32)
    region_idx = np.asarray(region_idx)
    neg = _neg_block()
    llw = _llw()
    # Fold the 0/1 masks into the weights once (exact in bf16).
    W1m = np.asarray(W1, np.float32) * np.asarray(M1, np.float32)
    W2m = np.asarray(W2, np.float32) * np.asarray(M2, np.float32)
    W3m = np.asarray(W3, np.float32) * np.asarray(M3, np.float32)
    f8 = ml_dtypes.float8_e4m3
    in_maps = []
    for r in range(N_CORES):
        xr = x[:n_total, region_idx[r]]  # [n, D]
        xt = np.ascontiguousarray(xr.T)  # [D, n]
        # fp8 hi/lo split of x, stacked [x_hi; x_lo] and replicated on the
        # two 64-row groups; both DoubleRow pair slots carry the same data.
        xhi = xt.astype(f8).astype(np.float32)
        xlo = (xt - xhi).astype(f8)
        xstk = np.concatenate([xhi.astype(f8), xlo], axis=0)  # [64, n]
        xt8 = np.empty((128, 2, n_total), f8)
        xt8[0:64, 0] = xstk
        xt8[0:64, 1] = xstk
        xt8[64:128] = xt8[0:64]

        # W1 fp8 hi/lo pair blocks: [64 rows, 2 pairs, H] per net.
        w18 = np.zeros((128, 8, 2, 128), f8)
        for g in range(4):
            for bp in range(4):
                w = W1m[r][4 * g + bp]  # [D, H] fp32
                whi = w.astype(f8).astype(np.float32)
                wlo = (w - whi).astype(f8)
                rb, cb = bp % 2, bp // 2
                rows = slice(64 * rb, 64 * rb + 64)
                blk = np.zeros((64, 2, 128), f8)
                blk[0:32, 0] = whi.astype(f8)
                blk[0:32, 1] = wlo
                blk[32:64, 0] = whi.astype(f8)
                # blk[32:64, 1] stays 0
                w18[rows, 2 * g + cb] = blk

        wall = np.zeros((128, WALL_C), np.float32)
        wall[:, 0:128] = neg
        for g in range(4):
            base = 128 + g * GBLK
            # w2: concat over bp of [H, H] (lhsT: partition = h_in)
            wall[:, base : base + 512] = np.concatenate(
                [W2m[r][4 * g + bp] for bp in range(4)], axis=1
            )
            # w3 per net [H, 64] = [shift cols (0::2) | ls cols (1::2)]
            cols = []
            for bp in range(4):
                wb = W3m[r][4 * g + bp]  # [H, 2D]
                cols.append(np.concatenate([wb[:, 0::2], wb[:, 1::2]], axis=1))
            wall[:, base + 512 : base + 768] = np.concatenate(cols, axis=1)

        in_maps.append(
            {
                "wall": wall.astype(ml_dtypes.bfloat16),
                "llw": llw,
                "w18": w18,
                "xt8": xt8,
                "xts": xt.astype(ml_dtypes.bfloat16),
            }
        )
    return in_maps


_NC_CACHE = {}


def run(x, W1, W2, W3, M1, M2, M3, region_idx, trace=False, n_total=N):
    if n_total not in _NC_CACHE:
        _NC_CACHE[n_total] = build_nc(n_total)
    nc = _NC_CACHE[n_total]
    in_maps = shard_inputs(x, W1, W2, W3, M1, M2, M3, region_idx, n_total)
    res = run_bass_kernel_spmd(
        nc, in_maps, core_ids=list(range(N_CORES)), trace=trace
    )
    out = np.empty((n_total, R, B), dtype=np.float32)
    for r in range(N_CORES):
        o = res.results[r]["out"]  # [n_chunks, 16, CHUNK]
        out[:, r, :] = o.transpose(0, 2, 1).reshape(n_total, B)
    return out, res


def kernel(x, W1, W2, W3, M1, M2, M3, region_idx):
    out, _ = run(x, W1, W2, W3, M1, M2, M3, region_idx)
    return out


# revision 28
# speedup vs baseline: 1.0817x; 1.0817x over previous
"""Trainium2 Bass kernel for an autoregressive-flow (MAF) layer.

Reference computation (per region r, batch-network b):
    xr[n, d]   = x[n, region_idx[r, d]]                      # [N, D]
    h1 = relu(xr @ (W1*M1)[r,b])                             # [N, H]
    h2 = relu(h1 @ (W2*M2)[r,b])                             # [N, H]
    o  = h2 @ (W3*M3)[r,b]                                   # [N, 2D]
    shift = o[:, 0::2]; log_scale = o[:, 1::2]
    u  = (xr - shift) * exp(-log_scale)
    ll[n, r, b] = sum_d(-0.5*u^2 - 0.5*log(2*pi) - log_scale)

Sharding: region axis R=8 across the 8 NeuronCores; each core handles its
region's B=16 networks over all N=2048 samples.

Device dataflow (per core, "transposed" orientation), v3:
    - Masks are folded into the weights on the host (M is 0/1 so
      bf16(W*M) == bf16(W)*M exactly); the wall tensor carries only the
      masked weights -> half the weight DMA and no on-device mask pass.
    - HAM warmup: ~20 short fp32 matmuls on a memset tile issue as the
      very first PE work (during the input DMA) so the PE clock gate is
      at 2.4 GHz (K=8/8) before the first real matmul.
    - xtb [128, 2048] bf16: x-slice transposed, replicated on 4 partition
      row-groups (feeds 4x row-packed K=32 L1 matmuls + the seed matmul).
    - Per (chunk, group) work item: L1 (4 row-packed K=32 matmuls),
      relu-moves to bf16 SBUF split across ACT/DVE, L2 (4 full matmuls),
      relu-moves, L3 split into shift/log_scale halves with 4 networks
      column-packed per PSUM tile; shift tile seeded with -x via a
      negated tiled-identity matmul so PSUM holds (shift - x).
    - Tail: A = 0.5*(shift-x)^2 [ACT Square, scale=sqrt(0.5), bf16],
      B = exp(-2*ls) [ACT Exp, bf16], c = A*B [Pool bf16],
      v = c + ls [DVE -> fp32r].  One ll matmul per group (-1 block
      weights, fp32r) accumulates -(0.5*u^2 + ls) into a [16, 512] PSUM
      tile; bias add on DVE.
    - Emission of L3/ll is staggered one work item behind L1/L2 so the
      PE FIFO never head-of-line blocks on the relu-move/tail chains.
"""

import ml_dtypes
import numpy as np

import concourse.bacc as bacc
import concourse.mybir as mybir
from concourse.bass_utils import run_bass_kernel_spmd
from concourse.tile import TileContext

R, B, D, H, N, F = 8, 16, 32, 128, 2048, 256
HALF_LOG_2PI = 0.9189385332046727
N_CORES = 8
CHUNK = 512
F32 = mybir.dt.float32
F32R = mybir.dt.float32r
BF16 = mybir.dt.bfloat16

# wall column layout (bf16): [neg(128) | 4 x group-block(896)]
#   group-block: w1(128) w2(512) w3(256)   (masks pre-folded on host)
GBLK = 896
WALL_C = 128 + 4 * GBLK
SQRT_HALF = float(np.sqrt(0.5))
N_WARMUP = 9


def _neg_block():
    # Negated tiled identity: lhsT [32, 128], out rows 32*bp + d get -x_d.
    neg = np.zeros((128, 128), np.float32)
    for m in range(128):
        neg[m % D, m] = -1.0
    return neg


def _llw():
    # ll weights [128, 4, 16] fp32: for group g, col j = 4g+bp sums rows
    # 32bp..32bp+31 with -1 (v already holds 0.5*u^2 + ls).
    w = np.zeros((128, 4, 16), np.float32)
    for g in range(4):
        for bp in range(4):
            w[32 * bp : 32 * (bp + 1), g, 4 * g + bp] = -1.0
    return w.reshape(128, 64)


def build_nc(n_total=N):
    assert n_total % CHUNK == 0
    n_chunks = n_total // CHUNK
    n_items = 4 * n_chunks  # (chunk, group) work items

    nc = bacc.Bacc(
        "TRN2",
        target_bir_lowering=False,
        debug=False,
        enable_asserts=False,
        num_devices=N_CORES,
    )

    wall_d = nc.declare_dram_parameter("wall", [128, WALL_C], BF16, isOutput=False)
    llw_d = nc.declare_dram_parameter("llw", [128, 64], F32, isOutput=False)
    xt4_d = nc.declare_dram_parameter("xt4", [128, n_total], BF16, isOutput=False)
    out_d = nc.declare_dram_parameter("out", [n_chunks, 16, CHUNK], F32, isOutput=True)

    with TileContext(nc) as tc:
        with (
            tc.tile_pool(name="const", bufs=1) as cpool,
            tc.tile_pool(name="stage", bufs=1) as spool,
            tc.tile_pool(name="s1", bufs=8) as s1pool,
            tc.tile_pool(name="s2", bufs=8) as s2pool,
            tc.tile_pool(name="tail", bufs=2) as tpool,
            tc.tile_pool(name="vout", bufs=2) as vpool,
            tc.tile_pool(name="llo", bufs=2) as opool,
            tc.tile_pool(name="p1", bufs=3, space="PSUM") as p1pool,
            tc.tile_pool(name="p2", bufs=2, space="PSUM") as p2pool,
            tc.tile_pool(name="pt", bufs=1, space="PSUM") as ptpool,
            tc.tile_pool(name="pl", bufs=1, space="PSUM") as plpool,
            tc.tile_pool(name="pll", bufs=1, space="PSUM") as pllpool,
        ):
            wall = cpool.tile([128, WALL_C], BF16, tag="wall")
            llwst = spool.tile([128, 64], F32, tag="llwst")
            llwr = cpool.tile([128, 64], F32R, tag="llwr")
            xtb = cpool.tile([128, n_total], BF16, tag="xtb")
            wtile = cpool.tile([128, 64], F32, tag="warm")

            # HAM warmup: memset a small tile, then back-to-back short fp32
            # matmuls keep the PE busy from ~6.3us (while input DMA runs) so
            # the clock gate un-throttles before the first real matmul.
            ftile = cpool.tile([128, 256], BF16, tag="fill")
            nc.gpsimd.memset(wtile[:], 0.0)
            nc.gpsimd.memset(ftile[:], 0.0)
            # Warmup/filler matmuls write a scratch tile in the pll bank: that
            # bank has no real writes until the first ll matmul (emitted at
            # loop i=1), and that matmul's start=True overwrites the region.
            wps = pllpool.tile([16, CHUNK], F32, name="warmps", tag="llps")
            for _ in range(N_WARMUP):
                nc.tensor.matmul(
                    wps[0:16, 0:64],
                    wtile[:, 0:16],
                    wtile[:, 0:64],
                    start=True,
                    stop=True,
                )

            def emit_fill(n):
                # Short bf16 matmuls with no data deps: they run whenever the
                # PE would otherwise idle waiting on relu-moves, keeping the
                # HAM activity window busy during early pipe-fill so the clock
                # gate stays at K=8/8.  Only legal before the first emit_ll.
                for _ in range(n):
                    nc.tensor.matmul(
                        wps[0:16, 0:256],
                        ftile[:, 0:16],
                        ftile[:, 0:256],
                        start=True,
                        stop=True,
                    )

            # Input DMA spread across engine queues so the transfers run in
            # parallel (a single queue moves only ~90 GB/s): the first-needed
            # pieces (neg + L1 g0 weights, x chunk 0, rest of g0) go on their
            # own queues; bulk weights and x chunks 1-3 follow on sync.
            nc.scalar.dma_start(out=wall[:, 128:256], in_=wall_d[:, 128:256])
            nc.sync.dma_start(out=xtb[0:64, :CHUNK], in_=xt4_d[0:64, :CHUNK])
            nc.sync.dma_start(out=xtb[64:128, :CHUNK], in_=xt4_d[64:128, :CHUNK])
            nc.scalar.dma_start(out=wall[:, 256:1024], in_=wall_d[:, 256:1024])
            nc.scalar.dma_start(out=wall[:, 0:128], in_=wall_d[:, 0:128])
            nc.sync.dma_start(out=llwst[:], in_=llw_d[:])
            nc.scalar.dma_start(out=wall[:, 1024:1920], in_=wall_d[:, 1024:1920])
            nc.sync.dma_start(out=wall[:, 1920:], in_=wall_d[:, 1920:])
            nc.sync.dma_start(
                out=xtb[:, CHUNK : 2 * CHUNK], in_=xt4_d[:, CHUNK : 2 * CHUNK]
            )
            nc.sync.dma_start(
                out=xtb[:, 2 * CHUNK : 3 * CHUNK], in_=xt4_d[:, 2 * CHUNK : 3 * CHUNK]
            )
            nc.sync.dma_start(out=xtb[:, 3 * CHUNK :], in_=xt4_d[:, 3 * CHUNK :])

            neg = wall[0:D, 0:128]

            nc.vector.tensor_copy(out=llwr[:], in_=llwst[:])

            def w1m(g):
                base = 128 + g * GBLK
                return wall[:, base : base + 128]

            def w2m(b):
                g, bp = b // 4, b % 4
                base = 128 + g * GBLK + 128 + 128 * bp
                return wall[:, base : base + 128]

            def w3m(b, half):
                # half 0 = shift cols, 1 = log-scale cols
                g, bp = b // 4, b % 4
                base = 128 + g * GBLK + 640 + 64 * bp + 32 * half
                return wall[:, base : base + 32]

            # Per-item state carried between staggered emission phases.
            st = [None] * n_items

            def item_cg(i):
                return i // 4, i % 4

            def emit_L1(i, bps):
                c, g = item_cg(i)
                cs = slice(c * CHUNK, (c + 1) * CHUNK)
                s = st[i]
                for bp in bps:
                    prow = slice(32 * bp, 32 * (bp + 1))
                    p1 = p1pool.tile([128, CHUNK], F32, tag="p1")
                    nc.tensor.matmul(
                        p1[:],
                        w1m(g)[prow, :],
                        xtb[prow, cs],
                        start=True,
                        stop=True,
                        tile_position=(32 * bp, 0),
                    )
                    s["p1"][bp] = p1

            def emit_s1_moves(i):
                # bp -> engine: [ACT, DVE, ACT, DVE]
                s = st[i]
                for bp in range(4):
                    s1 = s1pool.tile([128, CHUNK], BF16, tag="s1")
                    p1 = s["p1"][bp]
                    if bp in (0, 2):
                        nc.scalar.activation(
                            s1[:], p1[:], mybir.ActivationFunctionType.Relu
                        )
                    else:
                        nc.vector.tensor_scalar_max(s1[:], p1[:], 0.0)
                    s["s1"][bp] = s1

            def emit_L2(i):
                c, g = item_cg(i)
                s = st[i]
                for bp in range(4):
                    b = 4 * g + bp
                    p2 = p2pool.tile([128, CHUNK], F32, tag="p2")
                    nc.tensor.matmul(
                        p2[:],
                        w2m(b),
                        s["s1"][bp][:],
                        start=True,
                        stop=True,
                    )
                    s["p2"][bp] = p2

            def emit_s2_moves(i):
                # bp -> engine: [DVE, ACT, DVE, ACT]
                s = st[i]
                for bp in range(4):
                    s2 = s2pool.tile([128, CHUNK], BF16, tag="s2")
                    p2 = s["p2"][bp]
                    if bp in (1, 3):
                        nc.scalar.activation(
                            s2[:], p2[:], mybir.ActivationFunctionType.Relu
                        )
                    else:
                        nc.vector.tensor_scalar_max(s2[:], p2[:], 0.0)
                    s["s2"][bp] = s2

            def emit_L3_shift(i):
                c, g = item_cg(i)
                cs = slice(c * CHUNK, (c + 1) * CHUNK)
                s = st[i]
                tps = ptpool.tile([128, CHUNK], F32, tag="tps")
                nc.tensor.matmul(
                    tps[:],
                    neg,
                    xtb[0:D, cs],
                    start=True,
                    stop=False,
                    skip_group_check=True,
                    tile_position=(0, 0),
                )
                for bp in range(4):
                    b = 4 * g + bp
                    nc.tensor.matmul(
                        tps[32 * bp : 32 * (bp + 1), :],
                        w3m(b, 0),
                        s["s2"][bp][:],
                        start=False,
                        stop=(bp == 3),
                        skip_group_check=True,
                        tile_position=(0, 32 * bp),
                    )
                s["tps"] = tps

            def emit_L3_ls(i, bps):
                c, g = item_cg(i)
                s = st[i]
                if s.get("lps") is None:
                    s["lps"] = plpool.tile([128, CHUNK], F32, name="lps", tag="lps")
                lps = s["lps"]
                for bp in bps:
                    b = 4 * g + bp
                    nc.tensor.matmul(
                        lps[32 * bp : 32 * (bp + 1), :],
                        w3m(b, 1),
                        s["s2"][bp][:],
                        start=True,
                        stop=True,
                        tile_position=(0, 32 * bp),
                    )

            def emit_tail(i):
                s = st[i]
                a_sb = tpool.tile([128, CHUNK], BF16, tag="a")
                nc.scalar.activation(
                    a_sb[:],
                    s["tps"][:],
                    mybir.ActivationFunctionType.Square,
                    scale=SQRT_HALF,
                )
                b_sb = tpool.tile([128, CHUNK], BF16, tag="b")
                nc.scalar.activation(
                    b_sb[:],
                    s["lps"][:],
                    mybir.ActivationFunctionType.Exp,
                    scale=-2.0,
                )
                c_sb = tpool.tile([128, CHUNK], BF16, tag="c")
                nc.vector.tensor_mul(out=c_sb[:], in0=a_sb[:], in1=b_sb[:])
                v = vpool.tile([128, CHUNK], F32R, tag="v")
                nc.vector.tensor_add(out=v[:], in0=c_sb[:], in1=s["lps"][:])
                s["v"] = v

            llps_ref = [None]

            def emit_ll(i):
                c, g = item_cg(i)
                if g == 0:
                    llps_ref[0] = pllpool.tile([16, CHUNK], F32, name="llps", tag="llps")
                llps = llps_ref[0]
                nc.tensor.matmul(
                    llps[:],
                    llwr[:, 16 * g : 16 * (g + 1)],
                    st[i]["v"][:],
                    start=(g == 0),
                    stop=(g == 3),
                    skip_group_check=True,
                )
                if g == 3:
                    ll_sb = opool.tile([16, CHUNK], F32, tag="ll")
                    nc.scalar.activation(
                        ll_sb[:],
                        llps[:],
                        mybir.ActivationFunctionType.Copy,
                        bias=float(-D * HALF_LOG_2PI),
                    )
                    nc.sync.dma_start(out=out_d[c], in_=ll_sb[:])
                st[i]["v"] = None
                st[i] = None

            # Staggered emission: PE slot i runs item i's L3 + item (i-1)'s
            # ll interleaved with item (i+1)'s L1/L2, ordered so the PE FIFO
            # never head-of-line blocks on relu-move/tail chains or PSUM
            # bank reuse (p1/p2 are double-buffered; each matmul that reuses
            # a bank issues well after the relu move that frees it).
            def start_item(i):
                st[i] = {"p1": [None] * 4, "s1": [None] * 4,
                         "p2": [None] * 4, "s2": [None] * 4, "lps": None}

            start_item(0)
            emit_L1(0, range(4))
            emit_fill(8)
            emit_s1_moves(0)
            emit_L2(0)
            emit_fill(4)
            emit_s2_moves(0)
            for i in range(n_items):
                nxt = i + 1
                if nxt < n_items:
                    start_item(nxt)
                    emit_L1(nxt, [0, 1, 2])
                if i == 0:
                    emit_fill(3)
                emit_L3_shift(i)
                if i == 0:
                    emit_fill(3)
                if nxt < n_items:
                    emit_L1(nxt, [3])
                    emit_s1_moves(nxt)
                emit_L3_ls(i, [0, 1, 2, 3])
                emit_tail(i)
                if i >= 1:
                    emit_ll(i - 1)
                if nxt < n_items:
                    emit_L2(nxt)
                    emit_s2_moves(nxt)
                if i == 0:
                    emit_fill(3)
            emit_ll(n_items - 1)

    nc.compile()
    return nc


def shard_inputs(x, W1, W2, W3, M1, M2, M3, region_idx, n_total=N):
    """Per-core input dicts: pure gather/transpose/replicate layout prep."""
    x = np.asarray(x, dtype=np.float32)
    region_idx = np.asarray(region_idx)
    neg = _neg_block()
    llw = _llw()
    # Fold the 0/1 masks into the weights once (exact in bf16).
    W1m = np.asarray(W1, np.float32) * np.asarray(M1, np.float32)
    W2m = np.asarray(W2, np.float32) * np.asarray(M2, np.float32)
    W3m = np.asarray(W3, np.float32) * np.asarray(M3, np.float32)
    in_maps = []
    for r in range(N_CORES):
        xr = x[:n_total, region_idx[r]]  # [n, D]
        xt = np.ascontiguousarray(xr.T)  # [D, n]
        xt4 = np.ascontiguousarray(np.tile(xt, (4, 1)))  # [128, n]

        wall = np.zeros((128, WALL_C), np.float32)
        wall[:, 0:128] = neg
        for g in range(4):
            base = 128 + g * GBLK

            # w1: [4, D, H] for nets 4g..4g+3 -> rows 32*bp + d
            wall[:, base : base + 128] = (
                W1m[r].reshape(4, 4, D, H)[g].reshape(128, H)
            )
            # w2: concat over bp of [H, H] (lhsT: partition = h_in)
            wall[:, base + 128 : base + 640] = np.concatenate(
                [W2m[r][4 * g + bp] for bp in range(4)], axis=1
            )

            # w3 per net [H, 64] = [shift cols (0::2) | ls cols (1::2)]
            cols = []
            for bp in range(4):
                wb = W3m[r][4 * g + bp]  # [H, 2D]
                cols.append(np.concatenate([wb[:, 0::2], wb[:, 1::2]], axis=1))
            wall[:, base + 640 : base + 896] = np.concatenate(cols, axis=1)

        in_maps.append(
            {
                "wall": wall.astype(ml_dtypes.bfloat16),
                "llw": llw,
                "xt4": xt4.astype(ml_dtypes.bfloat16),
            }
        )
    return in_maps


_NC_CACHE = {}


def run(x, W1, W2, W3, M1, M2, M3, region_idx, trace=False, n_total=N):
    if n_total not in _NC_CACHE:
        _NC_CACHE[n_total] = build_nc(n_total)
    nc = _NC_CACHE[n_total]
    in_maps = shard_inputs(x, W1, W2, W3, M1, M2, M3, region_idx, n_total)
    res = run_bass_kernel_spmd(
        nc, in_maps, core_ids=list(range(N_CORES)), trace=trace
    )
    out = np.empty((n_total, R, B), dtype=np.float32)
    for r in range(N_CORES):
        o = res.results[r]["out"]  # [n_chunks, 16, CHUNK]
        out[:, r, :] = o.transpose(0, 2, 1).reshape(n_total, B)
    return out, res


def kernel(x, W1, W2, W3, M1, M2, M3, region_idx):
    out, _ = run(x, W1, W2, W3, M1, M2, M3, region_idx)
    return out
